# revision 1
# baseline (speedup 1.0000x reference)
"""Trainium2 Bass kernel for a bilinear decoder:

    u = z_user[row]; m = z_movie[col]                      # [E, 64] gathers
    logits[e, r] = u_e^T Q_r m_e                           # [E, 5]
    out = log_softmax(logits, axis=-1)

Strategy: data-parallel over edges on 8 NeuronCores (full z tables + Q
replicated, edge_label_index sharded). Per core, edges are processed in
tiles of P partitions x K edges-per-partition:
  - indirect SWDGE DMA gathers u/m rows, one 128-edge group per
    instruction (the HW consumes exactly one index per partition per
    indirect DMA; this instruction stream is the kernel's bottleneck at
    ~1.4us per instruction of GPSIMD descriptor-generation time)
  - PE transposes u groups of 128 edges to [64, 128] (two groups per
    transpose), ACT copies them PSUM->SBUF
  - PE matmul per group: W[e, (r,l)] = sum_k u_t[k,e] * Qflat[k, (r,l)]
  - DVE multiply W by gathered m (broadcast over r) + reduce over l
  - batched log_softmax over each tile, DMA out

Measured on 8 axon trn2 cores: rel err 2.4e-7 vs the jax reference,
HW exec time 2.76 ms. Faster gather paths were investigated and are
blocked by this container's toolchain: dma_gather (batched-index Q7
ucode) needs a GPSIMD library switch whose MODIFY_POOL_CONFIG
instruction this walrus rejects ("ISA wrong length"), as it does all
custom-DVE ops.
"""

import os
import numpy as np

import concourse.bass as bass
import concourse.mybir as mybir
import concourse.tile as tile
from concourse.masks import make_identity
from concourse.bass_utils import run_bass_kernel_spmd
from concourse.vector_clock import ScopedClock

F32 = mybir.dt.float32
I32 = mybir.dt.int32

N_USERS = 100000
N_MOVIES = 100000
E_TOTAL = 1000000
D = 64
R = 5
N_CORES = 8
K_MAX = 32

# ---------------------------------------------------------------------------
# Workaround: this walrus build rejects >1 sem wait on a single instruction
# ("Too many sync wait commands") and the TileContext final drain carries one
# wait per live semaphore. Split them across chained single-wait drains.
_MAX_WAITS = 1


def _patched_drain_and_barrier(self, tick_clock, wait_clock):
    nc = self.nc
    probe = nc.sync.drain()
    wait_clock.add_sem_waits(probe.ins, ScopedClock({None: tick_clock.global_clock}))
    si = probe.ins.sync_info
    if si is not None and len(si.on_wait) > _MAX_WAITS:
        waits = list(si.on_wait)
        del si.on_wait[_MAX_WAITS:]
        rest = waits[_MAX_WAITS:]
        while rest:
            chunk, rest = rest[:_MAX_WAITS], rest[_MAX_WAITS:]
            extra = nc.sync.drain()
            esi = extra.ins.sync_info
            if esi is None:
                extra.ins.sync_info = mybir.SyncInfo(on_wait=list(chunk), on_update=[])
            else:
                esi.on_wait.extend(chunk)
    nc.all_engine_barrier()
    assert self.sems is not None
    popped = nc._tile_sem_poison_stack.pop()
    assert popped is self._sem_poison
    nc.clear_and_free_semaphores(list(self.sems.allocated().values()))
    nc.all_engine_barrier()


tile.TileContext._drain_and_barrier = _patched_drain_and_barrier


def _split_multi_waits(nc, max_waits=_MAX_WAITS):
    """Walrus rejects instructions carrying more than one sem wait; move the
    extras onto single-wait NoOps inserted just before, on the same engine
    (program order on the engine preserves the wait-before-exec semantics)."""
    n_new = 0
    for fn in nc.m.functions:
        for bb in fn.blocks:
            insts = bb.instructions
            if not any(
                i.sync_info is not None and len(i.sync_info.on_wait) > max_waits
                for i in insts
            ):
                continue
            out = []
            for inst in insts:
                si = inst.sync_info
                if si is not None and len(si.on_wait) > max_waits:
                    waits = list(si.on_wait)
                    extra, keep = waits[:-max_waits], waits[-max_waits:]
                    for w in extra:
                        nop = mybir.InstNoOp(
                            name=f"{inst.name}_sw{n_new}",
                            sync_info=mybir.SyncInfo(on_wait=[w], on_update=[]),
                            bass_nofuse=True,
                            engine=inst.engine,
                        )
                        out.append(nop)
                        n_new += 1
                    del si.on_wait[:]
                    si.on_wait.extend(keep)
                out.append(inst)
            bb.instructions = out
    return n_new
# ---------------------------------------------------------------------------


# v2: fused multiply+cumulative-sum custom DVE op. One 1x pass over the
# [P, 5, 64] product computes running sums whose values at l=63 give the
# per-rating logits (via a first-difference afterwards), replacing the
# separate tensor_tensor multiply + tensor_reduce passes.
_MUL_CUMSUM = None


def _register_mul_cumsum():
    global _MUL_CUMSUM
    if _MUL_CUMSUM is not None:
        return _MUL_CUMSUM
    import concourse.dve_ops as dve_ops
    from concourse.dve_ops import DveOp, OPS, CUSTOM_DVE_SPECS
    from concourse.dve_spec import Spec, Src0, Src1, scan, AluOp, lower
    from concourse.dve_uop import DveOpSpec

    name = "MUL_CUMSUM_BK"
    for op in OPS:
        if op.name == name:
            _MUL_CUMSUM = op
            return op

    def _ref(in0, in1, s0, s1, imm2):
        p = in0.shape[0]
        a = np.asarray(in0, np.float32).reshape(p, -1)
        b = np.asarray(in1, np.float32).reshape(p, -1)
        return np.cumsum(a * b, axis=-1, dtype=np.float32).reshape(in0.shape)

    spec = Spec(body=scan(AluOp.ADD, Src0 * Src1), reference=_ref)
    row = dve_ops._CUSTOM_DVE_ROW_BASE + len(OPS)
    dve_ops._SUB_OPCODE_FOR_NAME[name] = row
    assert row < 0x20
    shas = {
        ver: DveOpSpec(
            name=name, opcode=row, uops=lower(spec, ver=ver), rd1_en=True
        ).sha(ver)
        for ver in ("v3", "v4")
    }
    op = DveOp(name, spec, subdim=False, uops_sha=shas)
    OPS.append(op)
    CUSTOM_DVE_SPECS[name] = spec
    _MUL_CUMSUM = op
    return op


# Custom-DVE ops fail walrus codegen in this container ("ISA wrong length"
# at visitInstISA) -- the fused multiply+scan path is kept for reference but
# off by default.
USE_SCAN = bool(int(os.environ.get("BK_SCAN", "0")))


def plan_tiles(e_c, k_max=K_MAX):
    """Split e_c edges into rectangles (base, P, K): P partitions x K edges."""
    tiles = []
    base = 0
    rem = e_c
    while rem >= 128:
        k = min(k_max, rem // 128)
        tiles.append((base, 128, k))
        base += 128 * k
        rem -= 128 * k
    if rem:
        tiles.append((base, rem, 1))
    return tiles


def build_nc(e_c, n_users=N_USERS, n_movies=N_MOVIES, k_max=K_MAX):
    nc = bass.Bass()
    z_user = nc.declare_dram_parameter("z_user", [n_users, D], F32, isOutput=False)
    z_movie = nc.declare_dram_parameter("z_movie", [n_movies, D], F32, isOutput=False)
    q_in = nc.declare_dram_parameter("q", [R, D, D], F32, isOutput=False)
    # edge_label_index int64 viewed host-side as int32 pairs (lo, hi)
    eli = nc.declare_dram_parameter("eli", [2, e_c, 2], I32, isOutput=False)
    out = nc.declare_dram_parameter("out", [e_c, R], F32, isOutput=True)

    tiles = plan_tiles(e_c, k_max)

    with tile.TileContext(nc) as tc:
        with (
            tc.tile_pool(name="const", bufs=1) as cpool,
            tc.tile_pool(name="io", bufs=5) as iopool,
            tc.tile_pool(name="work", bufs=3) as wpool,
            tc.tile_pool(name="ps_t", bufs=2, space="PSUM") as tpsum,
            tc.tile_pool(name="ps_w", bufs=3, space="PSUM") as wpsum,
        ):
            ident = cpool.tile([128, 128], F32)
            make_identity(nc, ident[:])
            # Qflat[k, (r, l)] = Q[r, k, l], replicated into both partition
            # halves so matmuls whose lhsT lives at partitions 64-127 can use
            # an rhs with a matching base partition.
            qsb = cpool.tile([128, R * 64], F32)
            for half in range(2):
                for r in range(R):
                    nc.sync.dma_start(
                        out=qsb[half * 64 : half * 64 + 64, r * 64 : (r + 1) * 64],
                        in_=q_in[r, :, :],
                    )

            for base, p, k in tiles:
                emit_tile(nc, tc, base, p, k, z_user, z_movie, eli, out,
                          qsb, ident, iopool, wpool, tpsum, wpsum)
    return nc


def emit_tile(nc, tc, base, p, k, z_user, z_movie, eli, out,
              qsb, ident, iopool, wpool, tpsum, wpsum):
    t = p * k

    # ---- index loads (int64 as int32 pairs) + compaction ----
    rowraw = iopool.tile([128, 2 * k], I32, tag="rowraw")
    colraw = iopool.tile([128, 2 * k], I32, tag="colraw")
    nc.sync.dma_start(
        out=rowraw[:p],
        in_=eli[0, base : base + t, :].rearrange("(p k) w -> p (k w)", p=p),
    )
    nc.sync.dma_start(
        out=colraw[:p],
        in_=eli[1, base : base + t, :].rearrange("(p k) w -> p (k w)", p=p),
    )
    rowi = iopool.tile([128, k], I32, tag="rowi")
    coli = iopool.tile([128, k], I32, tag="coli")
    nc.vector.tensor_copy(
        out=rowi[:p], in_=rowraw[:p].rearrange("p (k w) -> p k w", w=2)[:, :, 0]
    )
    nc.vector.tensor_copy(
        out=coli[:p], in_=colraw[:p].rearrange("p (k w) -> p k w", w=2)[:, :, 0]
    )

    # ---- gathers ----
    # HW indirect DMA consumes exactly one index per partition per
    # instruction (one contiguous run per partition), so gather one
    # 128-edge group at a time.
    u_t = iopool.tile([128, k * 64], F32, tag="u")
    m_t = iopool.tile([128, k * 64], F32, tag="m")
    for g in range(k):
        nc.gpsimd.indirect_dma_start(
            out=u_t[:p, g * 64 : (g + 1) * 64],
            out_offset=None,
            in_=z_user[:],
            in_offset=bass.IndirectOffsetOnAxis(ap=rowi[:p, g : g + 1], axis=0),
        )
        nc.gpsimd.indirect_dma_start(
            out=m_t[:p, g * 64 : (g + 1) * 64],
            out_offset=None,
            in_=z_movie[:],
            in_offset=bass.IndirectOffsetOnAxis(ap=coli[:p, g : g + 1], axis=0),
        )

    logits = wpool.tile([128, k * R], F32, tag="logits")
    if USE_SCAN:
        mul_cumsum = _register_mul_cumsum()
        sends = wpool.tile([128, k * R], F32, tag="sends")

    n_quads = (k + 3) // 4
    for qd in range(n_quads):
        g0 = 4 * qd
        ng = min(4, k - g0)  # groups in this quad
        tps = tpsum.tile([128, 256], F32, tag="tps")
        utq = wpool.tile([128, 256], F32, tag="utq")
        # transposes: pairs of groups -> [128, p] blocks in tps
        for h in range(0, ng, 2):
            w = min(2, ng - h) * 64
            nc.tensor.transpose(
                out=tps[:w, (h // 2) * 128 : (h // 2) * 128 + p],
                in_=u_t[:p, (g0 + h) * 64 : (g0 + h) * 64 + w],
                identity=ident[:p, :p],
            )
        if ng == 4 and p == 128:
            nc.scalar.copy(out=utq[:], in_=tps[:])
        else:
            for h in range(0, ng, 2):
                w = min(2, ng - h) * 64
                nc.scalar.copy(
                    out=utq[:w, (h // 2) * 128 : (h // 2) * 128 + p],
                    in_=tps[:w, (h // 2) * 128 : (h // 2) * 128 + p],
                )

        prodq = wpool.tile([128, 4 * R * 64], F32, tag="prodq")

        for ph in range(0, ng, 2):  # pairs within quad
            npair = min(2, ng - ph)
            wps = wpsum.tile([128, 1024], F32, tag="wps")
            for s in range(npair):
                g = g0 + ph + s
                lhsT = utq[
                    (s % 2) * 64 : (s % 2) * 64 + 64,
                    (ph // 2) * 128 : (ph // 2) * 128 + p,
                ]
                nc.tensor.matmul(
                    out=wps[:p, s * 512 : s * 512 + R * 64],
                    lhsT=lhsT,
                    rhs=qsb[(s % 2) * 64 : (s % 2) * 64 + 64, :],
                    start=True,
                    stop=True,
                )
            if USE_SCAN:
                # fused multiply + running sum per group; segment ends at
                # l=63 hold cumulative per-rating sums
                for s in range(npair):
                    g = g0 + ph + s
                    goff = (ph + s) * R * 64
                    nc.vector._custom_dve(
                        mul_cumsum,
                        out=prodq[:p, goff : goff + R * 64].rearrange(
                            "p (r l) -> p r l", l=64
                        ),
                        in0=wps[:p, s * 512 : s * 512 + R * 64].rearrange(
                            "p (r l) -> p r l", l=64
                        ),
                        in1=m_t[:p, g * 64 : (g + 1) * 64][:, None, :]
                        .to_broadcast([p, R, 64]),
                    )
            else:
                # multiply by m (broadcast over r); both pairs land in prodq
                w_ap = (
                    wps[:p]
                    .rearrange("p (s x) -> p s x", s=2)[:, :npair, : R * 64]
                    .rearrange("p s (r l) -> p s r l", l=64)
                )
                m_ap = (
                    m_t[:p, (g0 + ph) * 64 : (g0 + ph + npair) * 64]
                    .rearrange("p (s l) -> p s l", l=64)[:, :, None, :]
                    .to_broadcast([p, npair, R, 64])
                )
                prod_ap = prodq[
                    :p, ph * R * 64 : (ph + npair) * R * 64
                ].rearrange("p (s r l) -> p s r l", r=R, l=64)
                nc.vector.tensor_tensor(
                    out=prod_ap, in0=w_ap, in1=m_ap, op=mybir.AluOpType.mult
                )
        if not USE_SCAN:
            # one reduce over the whole quad
            nc.vector.tensor_reduce(
                out=logits[:p, g0 * R : (g0 + ng) * R].rearrange(
                    "p (s r) -> p s r", r=R
                ),
                in_=prodq[:p, : ng * R * 64].rearrange(
                    "p (s r l) -> p s r l", r=R, l=64
                ),
                axis=mybir.AxisListType.X,
                op=mybir.AluOpType.add,
            )
        if USE_SCAN:
            # pull the l=63 running sums: sends[p, g, r] = S[g, r]
            nc.vector.tensor_copy(
                out=sends[:p, g0 * R : (g0 + ng) * R],
                in_=prodq[:p, : ng * R * 64].rearrange(
                    "p (s r l) -> p s r l", r=R, l=64
                )[:, :, :, 63],
            )

    if USE_SCAN:
        # logits[g, 0] = S[g, 0]; logits[g, r] = S[g, r] - S[g, r-1]
        s3 = sends[:p].rearrange("p (k r) -> p k r", r=R)
        lg3w = logits[:p].rearrange("p (k r) -> p k r", r=R)
        nc.vector.tensor_copy(out=lg3w[:, :, 0:1], in_=s3[:, :, 0:1])
        nc.vector.tensor_tensor(
            out=lg3w[:, :, 1:R],
            in0=s3[:, :, 1:R],
            in1=s3[:, :, 0 : R - 1],
            op=mybir.AluOpType.subtract,
        )

    # ---- log_softmax over r (batched across the tile) ----
    mx = wpool.tile([128, k], F32, tag="mx")
    sm = wpool.tile([128, k], F32, tag="sm")
    ls = wpool.tile([128, k], F32, tag="ls")
    xm = wpool.tile([128, k * R], F32, tag="xm")
    ex = wpool.tile([128, k * R], F32, tag="ex")
    res = wpool.tile([128, k * R], F32, tag="res")

    lg3 = logits[:p].rearrange("p (k r) -> p k r", r=R)
    nc.vector.tensor_reduce(
        out=mx[:p], in_=lg3, axis=mybir.AxisListType.X, op=mybir.AluOpType.max
    )
    mx_b = mx[:p][:, :, None].to_broadcast([p, k, R])
    nc.vector.tensor_tensor(
        out=xm[:p].rearrange("p (k r) -> p k r", r=R),
        in0=lg3,
        in1=mx_b,
        op=mybir.AluOpType.subtract,
    )
    nc.scalar.activation(out=ex[:p], in_=xm[:p], func=mybir.ActivationFunctionType.Exp)
    nc.vector.tensor_reduce(
        out=sm[:p],
        in_=ex[:p].rearrange("p (k r) -> p k r", r=R),
        axis=mybir.AxisListType.X,
        op=mybir.AluOpType.add,
    )
    nc.scalar.activation(out=ls[:p], in_=sm[:p], func=mybir.ActivationFunctionType.Ln)
    # xm already has the max subtracted: log_softmax = xm - log(sum(exp(xm)))
    nc.vector.tensor_tensor(
        out=res[:p].rearrange("p (k r) -> p k r", r=R),
        in0=xm[:p].rearrange("p (k r) -> p k r", r=R),
        in1=ls[:p][:, :, None].to_broadcast([p, k, R]),
        op=mybir.AluOpType.subtract,
    )
    nc.sync.dma_start(
        out=out[base : base + t, :].rearrange("(p k) r -> p (k r)", p=p),
        in_=res[:p],
    )


_NC_CACHE = {}


def _get_nc(e_c, n_users, n_movies, k_max=K_MAX):
    key = (e_c, n_users, n_movies, k_max)
    if key not in _NC_CACHE:
        _NC_CACHE[key] = build_nc(e_c, n_users, n_movies, k_max)
    return _NC_CACHE[key]


def kernel(z_user, z_movie, edge_label_index, Q):
    z_user = np.ascontiguousarray(np.asarray(z_user, dtype=np.float32))
    z_movie = np.ascontiguousarray(np.asarray(z_movie, dtype=np.float32))
    Q = np.ascontiguousarray(np.asarray(Q, dtype=np.float32))
    eli = np.asarray(edge_label_index)
    e_total = eli.shape[1]
    assert e_total % N_CORES == 0
    e_c = e_total // N_CORES

    nc = _get_nc(e_c, z_user.shape[0], z_movie.shape[0])
    # hardware-only fixup (CoreSim can't model the inserted NoOps)
    n = _split_multi_waits(nc)
    if n:
        print(f"split {n} extra sem waits onto NoOps")

    # indices arrive as int64 (reference spec) or int32 (jax with x64 off);
    # marshal to int32 (lo, hi) pairs either way, values are < 2^31
    eli64 = np.ascontiguousarray(eli.astype(np.int64))
    eli32 = eli64.view(np.int32).reshape(2, e_total, 2)

    in_maps = []
    for c in range(N_CORES):
        sl = slice(c * e_c, (c + 1) * e_c)
        in_maps.append(
            {
                "z_user": z_user,
                "z_movie": z_movie,
                "q": Q,
                "eli": np.ascontiguousarray(eli32[:, sl, :]),
            }
        )

    trace = bool(int(os.environ.get("BK_TRACE", "0"))) and _ensure_ntff_hook()
    try:
        res = run_bass_kernel_spmd(nc, in_maps, list(range(N_CORES)), trace=trace)
    except Exception:
        if not trace:
            raise
        import traceback

        traceback.print_exc()
        print("trace path failed; re-running untraced")
        res = run_bass_kernel_spmd(nc, in_maps, list(range(N_CORES)), trace=False)
    if trace:
        print(f"HW exec time: {res.exec_time_ns} ns")
        kernel.last_exec_time_ns = res.exec_time_ns
        kernel.last_mean_exec_time_ns = res.mean_exec_time_ns
        kernel.last_results = res

    return np.concatenate([res.results[c]["out"] for c in range(N_CORES)], axis=0)


def _ensure_ntff_hook():
    """Register the axon NTFF profiling hook if the container didn't.

    trn_boot has a ctypes implementation but skips registration when
    `antenv.axon_hooks` is absent; synthesize that module so
    run_bass_kernel_spmd's trace branch can find the hook.
    """
    import sys
    import types

    try:
        from antenv.axon_hooks import get_axon_ntff_profile_hook  # noqa: F401

        return True
    except ImportError:
        pass
    try:
        from trn_agent_boot.trn_boot import _ntff_profile_via_ctypes

        hook = _ntff_profile_via_ctypes("/opt/axon/libaxon_pjrt.so")
    except Exception as e:
        print("ntff hook unavailable:", e)
        return False
    if hook is None:
        print("ntff hook unavailable: old libaxon_pjrt.so")
        return False
    mod = types.ModuleType("antenv.axon_hooks")
    state = {"hook": hook}
    mod.get_axon_ntff_profile_hook = lambda: state["hook"]
    mod.set_axon_ntff_profile_hook = lambda h: state.__setitem__("hook", h)
    sys.modules["antenv.axon_hooks"] = mod
    import antenv

    antenv.axon_hooks = mod
    return True



# revision 8
# speedup vs baseline: 2.2635x; 2.2635x over previous
"""Trainium2 Bass kernel for a bilinear decoder:

    u = z_user[row]; m = z_movie[col]                      # [E, 64] gathers
    logits[e, r] = u_e^T Q_r m_e                           # [E, 5]
    out = log_softmax(logits, axis=-1)

Strategy (v2): 2-D cell sharding + batched SWDGE dma_gather.

The SWDGE batched-gather ucode (InstDMAGatherAnt, mlp GPSIMD library,
compiled via Bacc which auto-inserts the library load) gathers up to
~1920 rows per instruction but its int16 indices address at most 32768
table rows.  So edges are sharded host-side into a 4x4 grid of
(user-chunk, movie-chunk) cells of 25000 rows each -- two cells per
core, sharing one z_user slice -- and every gather addresses a <=25000
row slice with int16 indices.  Tables are converted host-side to bf16
padded to 256 B rows (the gather's granularity).

Per 896-edge tile: two 896-index dma_gathers (round-robin over 4 SWDGE
queues; 57 descriptors each, two in flight per 128-descriptor ring),
4 pair-transposes on PE, one ACT copy, 4 matmuls against a block-diag
[Q;Q] bf16 rhs, bf16 DVE multiply+reduce, batched log_softmax.

Empirical limits (measured here): >1024 indices per gather overflows
the 128-descriptor/lane SWDGE ring and wedges the core; the Pool
engine serializes desc-gen at ~994 ns + ~1.2 ns/idx per instruction,
which is this kernel's wall (~600 us/core).
"""

import os
import numpy as np
import ml_dtypes

import concourse.bacc as bacc
import concourse.bass as bass
import concourse.mybir as mybir
import concourse.tile as tile
from concourse.masks import make_identity
from concourse.bass_utils import run_bass_kernel_spmd

F32 = mybir.dt.float32
BF16 = mybir.dt.bfloat16
I16 = mybir.dt.int16

N_ROWS = 100000
E_TOTAL = 1000000
D = 64
R = 5
N_CORES = 8

CHUNK = 25000            # table rows per cell slice (fits int16 indices)
GRID = 4                 # 4x4 grid of (user-chunk, movie-chunk) cells
CAP = 64512              # padded edges per cell; 72 tiles of 896
TILE_E = 896             # edges per gather instruction (57 descs/ring)
NT_CELL = CAP // TILE_E  # 72
GROUPS = TILE_E // 128   # 7 edge groups per tile
N_QUEUES = 4


def build_nc():
    nc = bacc.Bacc("TRN2", num_swdge_queues=N_QUEUES)
    zu = nc.dram_tensor("zu", [CHUNK, 2 * D], BF16, kind="ExternalInput")
    zm0 = nc.dram_tensor("zm0", [CHUNK, 2 * D], BF16, kind="ExternalInput")
    zm1 = nc.dram_tensor("zm1", [CHUNK, 2 * D], BF16, kind="ExternalInput")
    # int16 idx streams, 16-partition wrapped and replicated to 128 partitions
    idxu = [nc.dram_tensor(f"idxu{s}", [128, CAP // 16], I16, kind="ExternalInput")
            for s in range(2)]
    idxm = [nc.dram_tensor(f"idxm{s}", [128, CAP // 16], I16, kind="ExternalInput")
            for s in range(2)]
    # block-diag [[Qflat, 0], [0, Qflat]] with Qflat[k, (r, l)] = Q[r, k, l]
    qbd = nc.dram_tensor("qbd", [128, 2 * R * D], BF16, kind="ExternalInput")
    out = nc.dram_tensor("out", [2 * CAP, R], F32, kind="ExternalOutput")

    gather_i = 0
    with tile.TileContext(nc) as tc:
        with (
            tc.tile_pool(name="const", bufs=1) as cpool,
            tc.tile_pool(name="io", bufs=4) as iopool,
            tc.tile_pool(name="work", bufs=3) as wpool,
            tc.tile_pool(name="ps_t", bufs=2, space="PSUM") as tpsum,
            tc.tile_pool(name="ps_w", bufs=3, space="PSUM") as wpsum,
        ):
            ident = cpool.tile([128, 128], BF16)
            make_identity(nc, ident[:])
            qsb = cpool.tile([128, 2 * R * D], BF16)
            nc.sync.dma_start(out=qsb[:], in_=qbd[:])
            idxu_sb = [cpool.tile([128, CAP // 16], I16, name=f"idxu_sb{i}") for i in range(2)]
            idxm_sb = [cpool.tile([128, CAP // 16], I16, name=f"idxm_sb{i}") for i in range(2)]
            for s in range(2):
                nc.sync.dma_start(out=idxu_sb[s][:], in_=idxu[s][:])
                nc.sync.dma_start(out=idxm_sb[s][:], in_=idxm[s][:])

            for s in range(2):
                zm = zm0 if s == 0 else zm1
                for t in range(NT_CELL):
                    gather_i = emit_tile(
                        nc, s, t, zu, zm, idxu_sb[s], idxm_sb[s], qsb, ident,
                        out, iopool, wpool, tpsum, wpsum, gather_i)
    return nc


def emit_tile(nc, s, t, zu, zm, idxu_sb, idxm_sb, qsb, ident, out,
              iopool, wpool, tpsum, wpsum, gather_i):
    c0 = t * (TILE_E // 16)
    c1 = (t + 1) * (TILE_E // 16)

    # ---- gathers: [128, g, 128] bf16; idx i -> partition i%128, group i//128
    ut = iopool.tile([128, GROUPS, 128], BF16, tag="ut")
    mt = iopool.tile([128, GROUPS, 128], BF16, tag="mt")
    nc.gpsimd.dma_gather(
        out_ap=ut[:], in_ap=zu[:], idxs_ap=idxu_sb[:, c0:c1],
        num_idxs=TILE_E, num_idxs_reg=TILE_E, elem_size=2 * D,
        queue_num=gather_i % N_QUEUES)
    gather_i += 1
    nc.gpsimd.dma_gather(
        out_ap=mt[:], in_ap=zm[:], idxs_ap=idxm_sb[:, c0:c1],
        num_idxs=TILE_E, num_idxs_reg=TILE_E, elem_size=2 * D,
        queue_num=gather_i % N_QUEUES)
    gather_i += 1

    # ---- transposes: u^T per group, [64 l-partitions, 128 edge cols]
    n_pairs = (GROUPS + 1) // 2  # 4 (last pair is a single group)
    tps = tpsum.tile([128, GROUPS * 128], BF16, tag="tps")
    for g in range(GROUPS):
        nc.tensor.transpose(
            out=tps[:D, g * 128: g * 128 + 128],
            in_=ut[:, g, :D],
            identity=ident[:],
        )
    utq = wpool.tile([128, GROUPS * 128], BF16, tag="utq")
    nc.scalar.copy(out=utq[:D], in_=tps[:D])

    logits = wpool.tile([128, GROUPS * R], F32, tag="logits")

    for h in range(n_pairs):
        g0 = 2 * h
        ng = min(2, GROUPS - g0)
        # W chunks at 512-f32 (2 KB bank-aligned) offsets in PSUM
        wps = wpsum.tile([128, 1024], F32, tag="wps")
        # W[p, s*512 + (r, l)] = sum_k u^T[k, p] qflat[k, (r,l)]
        for sgl in range(ng):
            g = g0 + sgl
            nc.tensor.matmul(
                out=wps[:, sgl * 512: sgl * 512 + R * D],
                lhsT=utq[:D, g * 128: g * 128 + 128],
                rhs=qsb[:D, : R * D],
                start=True, stop=True,
            )
        wsb = wpool.tile([128, 640], BF16, tag="wsb")
        nc.scalar.copy(
            out=wsb[:, : ng * R * D].rearrange("p (s y) -> p s y", y=R * D),
            in_=wps[:].rearrange("p (s x) -> p s x", x=512)[:, :ng, : R * D])
        prod = wpool.tile([128, 640], BF16, tag="prod")
        w_ap = wsb[:, : ng * R * D].rearrange("p (s r l) -> p s r l", r=R, l=D)
        m_ap = (mt[:, g0: g0 + ng, :D]
                .rearrange("p s l -> p s l")[:, :, None, :]
                .to_broadcast([128, ng, R, D]))
        nc.vector.tensor_tensor(
            out=prod[:, : ng * R * D].rearrange("p (s r l) -> p s r l", r=R, l=D),
            in0=w_ap, in1=m_ap, op=mybir.AluOpType.mult)
        nc.vector.tensor_reduce(
            out=logits[:, g0 * R: (g0 + ng) * R].rearrange("p (s r) -> p s r", r=R),
            in_=prod[:, : ng * R * D].rearrange("p (s r l) -> p s r l", r=R, l=D),
            axis=mybir.AxisListType.X, op=mybir.AluOpType.add)

    # ---- batched log_softmax over r ----
    k = GROUPS
    mx = wpool.tile([128, k], F32, tag="mx")
    sm = wpool.tile([128, k], F32, tag="sm")
    ls = wpool.tile([128, k], F32, tag="ls")
    xm = wpool.tile([128, k * R], F32, tag="xm")
    ex = wpool.tile([128, k * R], F32, tag="ex")
    res = wpool.tile([128, k * R], F32, tag="res")
    lg3 = logits[:].rearrange("p (k r) -> p k r", r=R)
    nc.vector.tensor_reduce(
        out=mx[:], in_=lg3, axis=mybir.AxisListType.X, op=mybir.AluOpType.max)
    nc.vector.tensor_tensor(
        out=xm[:].rearrange("p (k r) -> p k r", r=R),
        in0=lg3, in1=mx[:][:, :, None].to_broadcast([128, k, R]),
        op=mybir.AluOpType.subtract)
    nc.scalar.activation(out=ex[:], in_=xm[:], func=mybir.ActivationFunctionType.Exp)
    nc.vector.tensor_reduce(
        out=sm[:], in_=ex[:].rearrange("p (k r) -> p k r", r=R),
        axis=mybir.AxisListType.X, op=mybir.AluOpType.add)
    nc.scalar.activation(out=ls[:], in_=sm[:], func=mybir.ActivationFunctionType.Ln)
    nc.vector.tensor_tensor(
        out=res[:].rearrange("p (k r) -> p k r", r=R),
        in0=xm[:].rearrange("p (k r) -> p k r", r=R),
        in1=ls[:][:, :, None].to_broadcast([128, k, R]),
        op=mybir.AluOpType.subtract)
    # edge (s, t, g, p) -> out row s*CAP + t*TILE_E + g*128 + p
    base = s * CAP + t * TILE_E
    nc.sync.dma_start(
        out=out[base: base + TILE_E, :].rearrange("(g p) r -> p g r", p=128),
        in_=res[:].rearrange("p (g r) -> p g r", r=R))
    return gather_i


_NC_CACHE = {}


def _get_nc():
    if "nc" not in _NC_CACHE:
        nc = build_nc()
        nc.finalize()
        _NC_CACHE["nc"] = nc
    return _NC_CACHE["nc"]


def _wrap16(a):
    """[n] int16 -> [128, n/16]: idx i at partition i%16, col i//16, x8 blocks."""
    w = a.reshape(-1, 16).T  # [16, n/16]
    return np.ascontiguousarray(np.tile(w, (8, 1)))


def kernel(z_user, z_movie, edge_label_index, Q):
    z_user = np.asarray(z_user, dtype=np.float32)
    z_movie = np.asarray(z_movie, dtype=np.float32)
    Q = np.asarray(Q, dtype=np.float32)
    eli = np.asarray(edge_label_index)
    row = eli[0].astype(np.int64)
    col = eli[1].astype(np.int64)
    e_total = row.shape[0]

    # bf16 tables padded to 256B rows
    zu_b = np.zeros((N_ROWS, 2 * D), dtype=ml_dtypes.bfloat16)
    zu_b[:, :D] = z_user.astype(ml_dtypes.bfloat16)
    zm_b = np.zeros((N_ROWS, 2 * D), dtype=ml_dtypes.bfloat16)
    zm_b[:, :D] = z_movie.astype(ml_dtypes.bfloat16)

    # block-diag [[Qflat, 0], [0, Qflat]] bf16, Qflat[k, (r,l)] = Q[r,k,l]
    qflat = np.transpose(Q, (1, 0, 2)).reshape(D, R * D)
    qbd = np.zeros((128, 2 * R * D), dtype=ml_dtypes.bfloat16)
    qbd[:D, : R * D] = qflat.astype(ml_dtypes.bfloat16)
    qbd[D:, R * D:] = qflat.astype(ml_dtypes.bfloat16)

    # 4x4 cell assignment
    cell = (row // CHUNK) * GRID + (col // CHUNK)
    order = np.argsort(cell, kind="stable")
    cell_sorted = cell[order]
    counts = np.bincount(cell_sorted, minlength=GRID * GRID)
    assert counts.max() <= CAP, f"cell overflow: {counts.max()} > {CAP}"
    starts = np.zeros(GRID * GRID + 1, dtype=np.int64)
    np.cumsum(counts, out=starts[1:])

    nc = _get_nc()

    in_maps = []
    gathers = []  # (core, slot, edge_ids) for unshard
    for c in range(N_CORES):
        a = c // 2
        bs = (2 * (c % 2), 2 * (c % 2) + 1)
        m = {
            "zu": np.ascontiguousarray(zu_b[a * CHUNK:(a + 1) * CHUNK]),
            "qbd": qbd,
        }
        for s, b in enumerate(bs):
            cid = a * GRID + b
            ids = order[starts[cid]:starts[cid + 1]]
            n = len(ids)
            iu = np.zeros(CAP, dtype=np.int16)
            im = np.zeros(CAP, dtype=np.int16)
            iu[:n] = (row[ids] - a * CHUNK).astype(np.int16)
            im[:n] = (col[ids] - b * CHUNK).astype(np.int16)
            m[f"zm{s}"] = np.ascontiguousarray(zm_b[b * CHUNK:(b + 1) * CHUNK])
            m[f"idxu{s}"] = _wrap16(iu)
            m[f"idxm{s}"] = _wrap16(im)
            gathers.append((c, s, ids))
        in_maps.append(m)

    trace = bool(int(os.environ.get("BK_TRACE", "0"))) and _ensure_ntff_hook()
    res = run_bass_kernel_spmd(nc, in_maps, list(range(N_CORES)), trace=trace)
    if trace:
        kernel.last_exec_time_ns = res.exec_time_ns
        kernel.last_mean_exec_time_ns = res.mean_exec_time_ns
        kernel.last_results = res

    out_full = np.empty((e_total, R), dtype=np.float32)
    for c, s, ids in gathers:
        rows_c = res.results[c]["out"][s * CAP: s * CAP + len(ids)]
        out_full[ids] = rows_c
    return out_full


def _ensure_ntff_hook():
    """Register the axon NTFF profiling hook if the container didn't."""
    import sys
    import types

    try:
        from antenv.axon_hooks import get_axon_ntff_profile_hook  # noqa: F401

        return True
    except ImportError:
        pass
    try:
        from trn_agent_boot.trn_boot import _ntff_profile_via_ctypes

        hook = _ntff_profile_via_ctypes("/opt/axon/libaxon_pjrt.so")
    except Exception as e:
        print("ntff hook unavailable:", e)
        return False
    if hook is None:
        print("ntff hook unavailable: old libaxon_pjrt.so")
        return False
    mod = types.ModuleType("antenv.axon_hooks")
    state = {"hook": hook}
    mod.get_axon_ntff_profile_hook = lambda: state["hook"]
    mod.set_axon_ntff_profile_hook = lambda h: state.__setitem__("hook", h)
    sys.modules["antenv.axon_hooks"] = mod
    import antenv

    antenv.axon_hooks = mod
    return True
